# revision 24
# baseline (speedup 1.0000x reference)
# Trainium2 Bass kernel: transformer encoder block, 8-way sequence-parallel.
#
# v2 design (from v1 trace analysis: ACT-bound softmax exp, late AllGather,
# half-empty PV matmuls):
#  * K/V AllGather is split into 4 head-group chunks (K bf16, V fp8e4m3),
#    each triggered as soon as its projection is staged, so attention pair 0
#    starts ~35us in instead of ~147us.
#  * softmax exp is spread over THREE engines: ACT computes true
#    exp(s/8 - 2) -> fp8 (the -2 shift cancels in normalization and keeps
#    e4m3 in range), DVE and GPSIMD compute a Schraudolph-style exp: the
#    e4m3 bit pattern of 2^y is round(8*y + 56 - sigma), evaluated as one
#    tensor_scalar (mult+add) with uint8 convert, written into the fp8 ex
#    tile via bitcast. HW-validated: DVE converts round-to-nearest.
#  * PV runs entirely in fp8 DoubleRow: contracts 256 keys per matmul at
#    0.5 cycles/row, with the ones-augmented V column producing the softmax
#    denominator in the same accumulation.
#  * head normalization happens per-pair inside the attention loop
#    (reciprocal_approx_fast + 2 broadcast matmuls) instead of a serial
#    phase-3 prologue; heads are written bf16 so the out-proj runs bf16.
#  * LayerNorm applies are fused to 3 scalar_tensor_tensor ops per tile.
import sys

if '/opt/trn_rl_repo' not in sys.path:
    sys.path.insert(0, '/opt/trn_rl_repo')

import numpy as np
import ml_dtypes

import concourse.bacc as bacc
import concourse.tile as tile
import concourse.mybir as mybir
from concourse.bass_utils import run_bass_kernel_spmd

F32 = mybir.dt.float32
F32R = mybir.dt.float32r
BF16 = mybir.dt.bfloat16
FP8 = mybir.dt.float8e4
U8 = mybir.dt.uint8
I16 = mybir.dt.int16
AF = mybir.ActivationFunctionType
ALU = mybir.AluOpType
DR = mybir.MatmulPerfMode.DoubleRow

E, H, DK, FF = 1024, 16, 64, 4096
EPS = 1e-5
P = 128

# ---- approximation knobs ----
V_FP8 = True          # gather V in fp8 + DoubleRow PV
GP_EXP = False        # gpsimd cannot read PSUM, so it cannot help with exp
# exp(s/8 - SHIFT) cancels in softmax normalization; SHIFT keeps the max
# exp value (max score/8 is ~8.0 for this data) under fp8e4's 240 ceiling
# (float8e4 here is IEEE e4m3: max normal 240, overflow -> inf).
EXP_SHIFT = 2.75
L2E = 1.4426950408889634
# DVE/GP Schraudolph (e4m3 bits): bits = round(s * A8 + B8), saturating u8
A8 = 0.125 * 8 * L2E
B8 = 8 * 7 - 0.7 - EXP_SHIFT * 8 * L2E
# int16/bf16 fallback constants (non-DR path)
A16 = 0.125 * 128 * L2E
B16 = 128 * 127 - 5.59 - EXP_SHIFT * 128 * L2E

TRACE = False
TRACE_KWARGS = {}
LAST_RESULT = None
DEBUG = False


def r32(ap):
    return ap.bitcast(F32R)


def exp_engine_pattern(gp_ok):
    # per-pair engine assignment for the 32 exp steps; weighted to the
    # engines' throughputs (ACT ~0.87, DVE ~0.84 with its other attention
    # work, GP ~0.5 steps/us) via greedy largest-remaining interleave
    counts = {'A': 15, 'D': 10, 'G': 7} if gp_ok else {'A': 16, 'D': 16}
    total = sum(counts.values())
    acc = {k: 0.0 for k in counts}
    out = []
    for _ in range(total):
        for k in counts:
            acc[k] += counts[k] / total
        pick = max(acc, key=lambda k: acc[k])
        acc[pick] -= 1.0
        out.append(pick)
    return out


def build_nc(s_total=4096, n_cores=8):
    sl = s_total // n_cores
    kte = E // P            # 8 contraction tiles over E
    skt = s_total // P      # 32 key tiles
    nkt2 = skt // 2         # 16 double-key blocks
    ffe = FF // P
    stl = sl // P           # 4 local key tiles
    kg = 16
    assert sl == 512

    nc = bacc.Bacc("TRN2", target_bir_lowering=False, debug=False,
                   num_devices=n_cores)

    def din(name, shape, dt=F32R):
        return nc.dram_tensor(name, shape, dt, kind="ExternalInput").ap()

    xT_d = din("xT", [P, kte, sl])
    xTb_d = din("xTb", [P, kte, sl], BF16)
    wq_d, wk_d = (din(n, [E // P, P, E], BF16) for n in ("Wq", "Wk"))
    wv_d = din("Wv", [4, P, kte * 256], BF16)       # 256-wide rhs tiles
    wo_d = din("Wo", [E // P, P, E], BF16)
    wf1_d = din("W_ff1", [FF // P, P, E], BF16)
    wf2_d = din("W_ff2", [kte, FF // (kg * P), P, kg * P], BF16)
    bq_d, bk_d, bo_d = (din(n, [P, E // P], F32) for n in ("bq", "bk", "bo"))
    bv_d = din("bv", [E])
    bf1_d = din("b_ff1", [P, FF // P], F32)
    bf2_d = din("b_ff2", [P, E // P], F32)
    g1_d, b1_d, g2_d, b2_d = (din(n, [P, E // P], F32)
                              for n in ("g1", "b1", "g2", "b2"))
    outT_d = nc.dram_tensor("outT", [E, sl], F32R, kind="ExternalOutput").ap()
    if DEBUG:
        dbg_q = nc.dram_tensor("dbg_q", [P, E // P, sl], BF16,
                               kind="ExternalOutput").ap()
        dbg_kth = nc.dram_tensor("dbg_kth", [P, s_total], BF16,
                                 kind="ExternalOutput").ap()
        dbg_va = nc.dram_tensor("dbg_va", [P, s_total // 256, 2, 160], U8,
                                kind="ExternalOutput").ap()
        dbg_ex = nc.dram_tensor("dbg_ex", [P, 2, 2, sl], U8,
                                kind="ExternalOutput").ap()
        dbg_dd = nc.dram_tensor("dbg_dd", [H // 2, 2, sl], F32,
                                kind="ExternalOutput").ap()
        dbg_heads = nc.dram_tensor("dbg_heads", [P, E // P, sl], BF16,
                                   kind="ExternalOutput").ap()
        dbg_y1 = nc.dram_tensor("dbg_y1", [P, E // P, sl], F32R,
                                kind="ExternalOutput").ap()

    # 4 gather chunks: chunk c = heads 4c..4c+3 (k rows 256c..256c+256 of the
    # transposed k; v cols 256c..256c+256), K bf16 and V fp8.
    VDT = FP8 if V_FP8 else BF16
    KCH = 256 * sl          # k elems per chunk per rank
    VCH = sl * 256          # v elems per chunk per rank
    ck_in = [nc.dram_tensor(f"ck_in{c}", [KCH], BF16).ap() for c in range(4)]
    ck_out = [nc.dram_tensor(f"ck_out{c}", [n_cores * KCH], BF16,
                             addr_space="Shared").ap() for c in range(4)]
    cv_in = [nc.dram_tensor(f"cv_in{c}", [VCH], VDT).ap() for c in range(4)]
    cv_out = [nc.dram_tensor(f"cv_out{c}", [n_cores * VCH], VDT,
                             addr_space="Shared").ap() for c in range(4)]
    rg = [list(range(n_cores))]

    pat = exp_engine_pattern(GP_EXP)
    pat0 = exp_engine_pattern(False)   # pair 0 avoids the gpsimd queue

    with tile.TileContext(nc) as tc, \
            nc.allow_low_precision(reason="bf16/fp8 attention pipeline"):
        with (
            tc.tile_pool(name="const", bufs=1) as cp,
            tc.tile_pool(name="acts", bufs=1) as ap_,
            tc.tile_pool(name="stat_sb", bufs=4) as statp,
        ):
            xt = cp.tile([P, kte, sl], F32R, tag="xt")
            nc.sync.dma_start(xt[:], xT_d[:])

            ones_f32 = cp.tile([P, 512], F32, tag="ones_f32")
            nc.vector.memset(ones_f32[:], 1.0)
            ones_row = cp.tile([1, 512], F32R, tag="ones_row")
            nc.vector.tensor_copy(ones_row[:], ones_f32[0:1, 0:512])
            ones_col = cp.tile([P, 1], F32R, tag="ones_col")
            nc.vector.tensor_copy(ones_col[:], ones_f32[:, 0:1])
            eps_t = cp.tile([1, 1], F32, tag="eps")
            nc.vector.memset(eps_t[:], EPS)
            shift_t = cp.tile([P, 1], F32, tag="shift")
            nc.vector.memset(shift_t[:], -EXP_SHIFT)

            def col_tile(d, n, tag):
                t = cp.tile([P, n // P], F32, tag=tag)
                nc.sync.dma_start(t[:], d[:])
                return t

            g1_c = col_tile(g1_d, E, "g1")
            b1_c = col_tile(b1_d, E, "b1")
            g2_c = col_tile(g2_d, E, "g2")
            b2_c = col_tile(b2_d, E, "b2")
            g1n = cp.tile([P, E // P], F32, tag="g1n")
            nc.vector.tensor_scalar(g1n[:], g1_c[:], -1.0, None, ALU.mult)
            g2n = cp.tile([P, E // P], F32, tag="g2n")
            nc.vector.tensor_scalar(g2n[:], g2_c[:], -1.0, None, ALU.mult)

            y1T = ap_.tile([P, kte, sl], F32R, tag="y1T")

            def ln_stat_accum(psmu, psvar, src_mt, mt, sqp):
                nc.tensor.matmul(psmu[:], r32(ones_col[:]), r32(src_mt),
                                 start=(mt == 0), stop=(mt == kte - 1))
                sq = sqp.tile([P, sl], F32R, tag="sq")
                nc.scalar.activation(sq[:], src_mt, AF.Square)
                nc.tensor.matmul(psvar[:], r32(ones_col[:]), r32(sq[:]),
                                 start=(mt == 0), stop=(mt == kte - 1))

            def ln_apply(psmu, psvar, src, res, g_c, b_c, gn_c, dst,
                         psp, sqp):
                """dst = res + LN(src) * g + b, 3 fused DVE ops per tile."""
                mu = statp.tile([1, sl], F32R, tag="stat_sb")
                nc.scalar.mul(mu[:], psmu[:], 1.0 / E)
                musq = statp.tile([1, sl], F32, tag="stat_sb")
                nc.vector.tensor_tensor(musq[:], mu[:], mu[:], ALU.mult)
                var = statp.tile([1, sl], F32, tag="stat_sb")
                nc.vector.tensor_scalar(var[:], psvar[:], 1.0 / E, None,
                                        ALU.mult)
                nc.vector.tensor_tensor(var[:], var[:], musq[:], ALU.subtract)
                sstd = statp.tile([1, sl], F32R, tag="stat_sb")
                nc.scalar.activation(sstd[:], var[:], AF.Sqrt,
                                     bias=eps_t[0:1, 0:1])
                rstd = statp.tile([1, sl], F32R, tag="stat_sb")
                nc.vector.reciprocal(rstd[:], sstd[:])
                murs = statp.tile([1, sl], F32R, tag="stat_sb")
                nc.vector.tensor_tensor(murs[:], mu[:], rstd[:], ALU.mult)
                psrb = psp.tile([P, sl], F32, tag="ps")
                nc.tensor.matmul(psrb[:], r32(ones_row[0:1, 0:P]), r32(rstd[:]),
                                 start=True, stop=True)
                psmrb = psp.tile([P, sl], F32, tag="ps")
                nc.tensor.matmul(psmrb[:], r32(ones_row[0:1, 0:P]),
                                 r32(murs[:]), start=True, stop=True)
                for mt in range(kte):
                    t = sqp.tile([P, sl], F32R, tag="sq")
                    nc.vector.scalar_tensor_tensor(t[:], src[:, mt, :],
                                                   g_c[:, mt:mt + 1], psrb[:],
                                                   ALU.mult, ALU.mult)
                    nc.vector.scalar_tensor_tensor(t[:], t[:],
                                                   b_c[:, mt:mt + 1],
                                                   res[:, mt, :],
                                                   ALU.add, ALU.add)
                    nc.vector.scalar_tensor_tensor(dst[:, mt, :], psmrb[:],
                                                   gn_c[:, mt:mt + 1], t[:],
                                                   ALU.mult, ALU.add)

            with tc.tile_pool(name="qh", bufs=1) as qhp:
                qT = qhp.tile([P, kte, sl], BF16, tag="qT")
                heads = qhp.tile([P, kte, sl], BF16, tag="heads")

                # ------------- phase 1: QKV + 4 chunked AllGathers ---------
                with (
                    tc.tile_pool(name="wcol", bufs=3) as wcp,
                    tc.tile_pool(name="wv_p", bufs=2) as wvp,
                    tc.tile_pool(name="kvstg", bufs=4) as stgp,
                    tc.tile_pool(name="rows1", bufs=1) as rp1,
                    tc.tile_pool(name="ps_qkv", bufs=3, space="PSUM") as psq,
                ):
                    xtb = wvp.tile([P, kte, sl], BF16, tag="xtb")
                    nc.sync.dma_start(xtb[:], xTb_d[:])
                    bq_c = rp1.tile([P, E // P], F32, tag="bq")
                    nc.sync.dma_start(bq_c[:], bq_d[:])
                    bk_c = rp1.tile([P, E // P], F32, tag="bk")
                    nc.sync.dma_start(bk_c[:], bk_d[:])
                    bv_r = rp1.tile([1, E], F32R, tag="bv")
                    nc.sync.dma_start(bv_r[:], bv_d.unsqueeze(0))

                    def q_mt(mt):
                        w = wcp.tile([P, kte, P], BF16, tag="wcol")
                        nc.sync.dma_start(w[:], wq_d[mt])
                        ps = psq.tile([P, sl], F32, tag="ps")
                        for kt in range(kte):
                            nc.tensor.matmul(ps[:], w[:, kt, :], xtb[:, kt, :],
                                             start=(kt == 0),
                                             stop=(kt == kte - 1))
                        nc.vector.tensor_scalar(qT[:, mt, :], ps[:],
                                                bq_c[:, mt:mt + 1], None,
                                                ALU.add)

                    def kv_chunk(c):
                        # k rows: mt tiles 2c, 2c+1
                        for j in range(2):
                            mt = 2 * c + j
                            w = wcp.tile([P, kte, P], BF16, tag="wcol")
                            nc.sync.dma_start(w[:], wk_d[mt])
                            ps = psq.tile([P, sl], F32, tag="ps")
                            for kt in range(kte):
                                nc.tensor.matmul(ps[:], w[:, kt, :],
                                                 xtb[:, kt, :],
                                                 start=(kt == 0),
                                                 stop=(kt == kte - 1))
                            stg = stgp.tile([P, sl], BF16, tag="kstg")
                            nc.vector.tensor_scalar(stg[:], ps[:],
                                                    bk_c[:, mt:mt + 1], None,
                                                    ALU.add)
                            nc.sync.dma_start(
                                ck_in[c][j * P * sl:(j + 1) * P * sl]
                                .rearrange("(r q) -> r q", q=sl), stg[:])
                        # v cols 256c..256c+256, all 512 local keys
                        wv = wvp.tile([P, kte, 256], BF16, tag="wv")
                        nc.sync.dma_start(wv[:], wv_d[c])
                        for st in range(stl):
                            ps = psq.tile([P, 256], F32, tag="psv")
                            for kt in range(kte):
                                nc.tensor.matmul(
                                    ps[:], xtb[:, kt, st * P:(st + 1) * P],
                                    wv[:, kt, :], start=(kt == 0), stop=False)
                            nc.tensor.matmul(ps[:], ones_row[0:1, 0:P],
                                             bv_r[0:1, c * 256:(c + 1) * 256],
                                             start=False, stop=True)
                            stg = stgp.tile([P, 256], VDT, tag="vstg")
                            nc.vector.tensor_copy(stg[:], ps[:])
                            nc.sync.dma_start(
                                cv_in[c][st * P * 256:(st + 1) * P * 256]
                                .rearrange("(r d) -> r d", d=256), stg[:])
                        nc.gpsimd.collective_compute(
                            "AllGather", ALU.bypass, replica_groups=rg,
                            ins=[ck_in[c].opt()], outs=[ck_out[c].opt()])
                        nc.gpsimd.collective_compute(
                            "AllGather", ALU.bypass, replica_groups=rg,
                            ins=[cv_in[c].opt()], outs=[cv_out[c].opt()])

                    kv_chunk(0)
                    q_mt(0)
                    q_mt(1)
                    kv_chunk(1)
                    q_mt(2)
                    q_mt(3)
                    kv_chunk(2)
                    q_mt(4)
                    q_mt(5)
                    kv_chunk(3)
                    for mt in range(6, kte):
                        q_mt(mt)

                # ------------- phase 2: attention ---------------------------
                with (
                    tc.tile_pool(name="attn", bufs=2) as atp,
                    tc.tile_pool(name="vau", bufs=2) as vap,
                    tc.tile_pool(name="exq", bufs=3) as exq,
                    tc.tile_pool(name="nrm", bufs=2) as nrm,
                    tc.tile_pool(name="ps_s", bufs=3, space="PSUM") as pss_p,
                    tc.tile_pool(name="ps_o", bufs=2, space="PSUM") as pso_p,
                ):
                    pend_norm = [None]

                    def emit_norm():
                        if pend_norm[0] is None:
                            return
                        hp, psoA, psoB, dd = pend_norm[0]
                        pend_norm[0] = None
                        if DEBUG:
                            nc.sync.dma_start(dbg_dd[hp:hp + 1, :, :], dd[:])
                        rr = nrm.tile([1, 2, sl], F32, tag="rr")
                        nc.vector.reciprocal_approx_fast(rr[:], dd[:])
                        rrr = nrm.tile([1, 2, sl], F32R, tag="rrr")
                        nc.scalar.copy(rrr[:], rr[:])
                        psb = pss_p.tile([P, 2, sl], F32, tag="pss")
                        nc.tensor.matmul(psb[0:DK, 0, :],
                                         r32(ones_row[0:1, 0:DK]),
                                         rrr[0:1, 0, :],
                                         start=True, stop=True)
                        nc.tensor.matmul(psb[0:DK, 1, :],
                                         r32(ones_row[0:1, 0:DK]),
                                         rrr[0:1, 1, :],
                                         start=True, stop=True)
                        sb_psb = nrm.tile([DK, 2, sl], F32, tag="sb_psb")
                        nc.scalar.copy(sb_psb[:], psb[0:DK, :, :])
                        nc.vector.tensor_tensor(heads[0:DK, hp, :],
                                                psoA[0:DK, :],
                                                sb_psb[:, 0, :], ALU.mult)
                        nc.vector.tensor_tensor(heads[DK:P, hp, :],
                                                psoB[0:DK, :],
                                                sb_psb[:, 1, :], ALU.mult)

                    for hp in range(H // 2):
                        c, ho = divmod(hp, 2)       # chunk, intra-chunk pair
                        kth = atp.tile([P, s_total], BF16, tag="kth")
                        for r in range(n_cores):
                            nc.sync.dma_start(
                                kth[:, r * sl:(r + 1) * sl],
                                ck_out[c][r * KCH + ho * P * sl:
                                          r * KCH + (ho + 1) * P * sl]
                                .rearrange("(p q) -> p q", q=sl))
                        # va2[p, kt2, i, d]: key kt2*256 + i*128 + p;
                        # head g block at d = g*80 .. g*80+65 (v | ones);
                        # 80-byte block stride keeps the DoubleRow ldweights
                        # K-slot step 16-aligned (ISA requirement).
                        va2 = vap.tile([P, nkt2, 2, 160], VDT, tag="va2")
                        for r in range(n_cores):
                            for g in range(2):
                                src = cv_out[c][r * VCH:(r + 1) * VCH] \
                                    .rearrange("(l i p d) -> p l i d",
                                               l=2, i=2, p=P) \
                                    [:, :, :, ho * P + g * DK:
                                     ho * P + (g + 1) * DK]
                                nc.sync.dma_start(
                                    va2[:, 2 * r:2 * r + 2, :,
                                        g * 80:g * 80 + DK], src)
                        for g in range(2):
                            nc.vector.tensor_copy(
                                va2[:, :, :, g * 80 + DK:g * 80 + DK + 1],
                                ones_f32[:, 0:2 * nkt2]
                                .rearrange("p (t g) -> p t g", g=2)
                                .unsqueeze(3))

                        if DEBUG and hp == 0:
                            nc.sync.dma_start(dbg_kth[:], kth[:])
                            nc.sync.dma_start(dbg_va[:], va2[:].bitcast(U8))
                        psoA = pso_p.tile([DK + 1, sl], F32, tag="pso")
                        psoB = pso_p.tile([DK + 1, sl], F32, tag="pso")
                        qA = qT[0:DK, hp, :]
                        qB = qT[DK:2 * DK, hp, :]
                        mypat = pat0 if hp == 0 else pat

                        ex_blocks = {}

                        def pv_block(b):
                            exb = ex_blocks.pop(b)
                            if DEBUG and hp == 0 and b == 0:
                                nc.sync.dma_start(dbg_ex[:],
                                                  exb[:].bitcast(U8))
                            nc.tensor.matmul(
                                psoA[:], va2[:, b, :, 0:DK + 1],
                                exb[:, :, 0, :],
                                start=(b == 0), stop=(b == nkt2 - 1),
                                perf_mode=DR, skip_group_check=True)
                            nc.tensor.matmul(
                                psoB[:], va2[:, b, :, 80:80 + DK + 1],
                                exb[:, :, 1, :],
                                start=(b == 0), stop=(b == nkt2 - 1),
                                perf_mode=DR, skip_group_check=True)

                        for kt in range(skt):
                            pss = pss_p.tile([P, 2, sl], F32, tag="pss")
                            nc.tensor.matmul(pss[:, 0, :],
                                             kth[0:DK, kt * P:(kt + 1) * P],
                                             qA, start=True, stop=True)
                            nc.tensor.matmul(pss[:, 1, :],
                                             kth[DK:2 * DK, kt * P:(kt + 1) * P],
                                             qB, start=True, stop=True)
                            if kt == 4:
                                emit_norm()
                            b, j = divmod(kt, 2)
                            if j == 0:
                                exb_new = exq.tile([P, 2, 2, sl], FP8,
                                                   tag="ex")
                                ex_blocks[b] = exb_new
                            exb = ex_blocks[b]
                            eng = mypat[kt]
                            if eng == 'A':
                                nc.scalar.activation(exb[:, j, :, :], pss[:],
                                                     AF.Exp,
                                                     bias=shift_t[:],
                                                     scale=0.125)
                            elif eng == 'D':
                                nc.vector.tensor_scalar(
                                    exb[:, j, :, :].bitcast(U8), pss[:],
                                    A8, B8, ALU.mult, ALU.add)
                            else:
                                nc.gpsimd.tensor_scalar(
                                    exb[:, j, :, :].bitcast(U8), pss[:],
                                    A8, B8, ALU.mult, ALU.add)
                            if kt % 2 == 1 and kt >= 5:
                                pv_block((kt - 5) // 2)
                        for b in (nkt2 - 2, nkt2 - 1):
                            pv_block(b)
                        # stash denominators in SBUF; normalization is
                        # deferred into the next pair's stream
                        dd = nrm.tile([1, 2, sl], F32, tag="dd")
                        nc.vector.tensor_copy(dd[0:1, 0, :],
                                              psoA[DK:DK + 1, :])
                        nc.vector.tensor_copy(dd[0:1, 1, :],
                                              psoB[DK:DK + 1, :])
                        pend_norm[0] = (hp, psoA, psoB, dd)
                    emit_norm()

                if DEBUG:
                    nc.sync.dma_start(dbg_q[:], qT[:])
                    nc.sync.dma_start(dbg_heads[:], heads[:])
                # ------------- phase 3: out-proj + LN1 ----------------------
                with (
                    tc.tile_pool(name="wo_p", bufs=3) as wop,
                    tc.tile_pool(name="rows3", bufs=1) as rp3,
                    tc.tile_pool(name="z_p", bufs=1) as zp,
                    tc.tile_pool(name="sq3", bufs=2) as sq3,
                    tc.tile_pool(name="ps_m3", bufs=3, space="PSUM") as psm3,
                    tc.tile_pool(name="ps_st3", bufs=2, space="PSUM") as pst3,
                ):
                    bo_c = rp3.tile([P, E // P], F32, tag="bo")
                    nc.sync.dma_start(bo_c[:], bo_d[:])
                    zT = zp.tile([P, kte, sl], F32R, tag="zT")
                    psmu1 = pst3.tile([1, sl], F32, tag="stat")
                    psvar1 = pst3.tile([1, sl], F32, tag="stat")
                    for mt in range(kte):
                        w = wop.tile([P, kte, P], BF16, tag="wo")
                        nc.sync.dma_start(w[:], wo_d[mt])
                        ps = psm3.tile([P, sl], F32, tag="ps")
                        for kt in range(kte):
                            nc.tensor.matmul(ps[:], w[:, kt, :],
                                             heads[:, kt, :],
                                             start=(kt == 0),
                                             stop=(kt == kte - 1))
                        nc.vector.tensor_scalar(zT[:, mt, :], ps[:],
                                                bo_c[:, mt:mt + 1], None,
                                                ALU.add)
                        ln_stat_accum(psmu1, psvar1, zT[:, mt, :], mt, sq3)
                    ln_apply(psmu1, psvar1, zT, xt, g1_c, b1_c, g1n, y1T,
                             psm3, sq3)
                    if DEBUG:
                        nc.sync.dma_start(dbg_y1[:], y1T[:])

            # ------------- phases 4-6: FF + LN2 ------------------------------
            with (
                tc.tile_pool(name="ff", bufs=1) as ffp,
                tc.tile_pool(name="wf1_p", bufs=3) as wf1p,
                tc.tile_pool(name="wf2_p", bufs=3) as wf2p,
                tc.tile_pool(name="sq4", bufs=2) as sq4,
                tc.tile_pool(name="ps_m4", bufs=3, space="PSUM") as psm4,
                tc.tile_pool(name="ps_st4", bufs=2, space="PSUM") as pst4,
            ):
                hT = ffp.tile([P, ffe, sl], BF16, tag="hT")
                ffT = ffp.tile([P, kte, sl], F32R, tag="ffT")
                y1b = ffp.tile([P, kte, sl], BF16, tag="y1b")
                for mt in range(kte):
                    nc.vector.tensor_copy(y1b[:, mt, :], y1T[:, mt, :])
                bf1_c = ffp.tile([P, FF // P], F32, tag="bf1")
                nc.sync.dma_start(bf1_c[:], bf1_d[:])
                bf2_c = ffp.tile([P, E // P], F32, tag="bf2")
                nc.sync.dma_start(bf2_c[:], bf2_d[:])
                for mt2 in range(ffe // 2):
                    wt = wf1p.tile([P, 2, kte, P], BF16, tag="wf1")
                    nc.sync.dma_start(
                        wt[:], wf1_d[2 * mt2:2 * mt2 + 2]
                        .rearrange("w p e -> p w e"))
                    for j in range(2):
                        mt = 2 * mt2 + j
                        ps = psm4.tile([P, sl], F32, tag="ps")
                        for kt in range(kte):
                            nc.tensor.matmul(ps[:], wt[:, j, kt, :],
                                             y1b[:, kt, :],
                                             start=(kt == 0),
                                             stop=(kt == kte - 1))
                        nc.vector.tensor_scalar(hT[:, mt, :], ps[:],
                                                bf1_c[:, mt:mt + 1], 0.0,
                                                ALU.add, ALU.max)
                psmu2 = pst4.tile([1, sl], F32, tag="stat")
                psvar2 = pst4.tile([1, sl], F32, tag="stat")
                for mt in range(kte):
                    ps = psm4.tile([P, sl], F32, tag="ps")
                    for g in range(ffe // kg):
                        wt2 = wf2p.tile([P, kg, P], BF16, tag="wf2")
                        nc.sync.dma_start(wt2[:], wf2_d[mt, g])
                        for j in range(kg):
                            kt = g * kg + j
                            nc.tensor.matmul(ps[:], wt2[:, j, :],
                                             hT[:, kt, :],
                                             start=(kt == 0),
                                             stop=(kt == ffe - 1))
                    nc.vector.tensor_scalar(ffT[:, mt, :], ps[:],
                                            bf2_c[:, mt:mt + 1], None,
                                            ALU.add)
                    ln_stat_accum(psmu2, psvar2, ffT[:, mt, :], mt, sq4)
                ln_apply(psmu2, psvar2, ffT, y1T, g2_c, b2_c, g2n, ffT,
                         psm4, sq4)
                for mt in range(kte):
                    nc.sync.dma_start(outT_d[mt * P:(mt + 1) * P, :],
                                      ffT[:, mt, :])

    nc.compile()
    return nc


def pretile_lhsT(W, dt):
    """[K, M] -> [M//128, 128, K]: out[mt, p, kt*128+m] = W[kt*128+p, mt*128+m]"""
    K, M = W.shape
    return np.ascontiguousarray(
        W.reshape(K // 128, 128, M // 128, 128).transpose(2, 1, 0, 3)
        .reshape(M // 128, 128, K).astype(dt))


def pretile_rhs(W, nw, dt):
    """[K, M] -> [M//nw, 128, (K//128)*nw]"""
    K, M = W.shape
    return np.ascontiguousarray(
        W.reshape(K // 128, 128, M // nw, nw).transpose(2, 1, 0, 3)
        .reshape(M // nw, 128, (K // 128) * nw).astype(dt))


def pretile_wf2(W, dt, kg=16):
    """[FF, E] -> [E//128, FF//(kg*128), 128, kg*128]"""
    K, M = W.shape
    return np.ascontiguousarray(
        W.reshape(K // (kg * 128), kg, 128, M // 128, 128)
        .transpose(3, 0, 2, 1, 4)
        .reshape(M // 128, K // (kg * 128), 128, kg * 128).astype(dt))


def pretile_x(xTs, dt):
    """[E, sl] -> [128, E//128, sl]"""
    Ed, sl = xTs.shape
    return np.ascontiguousarray(
        xTs.reshape(Ed // 128, 128, sl).transpose(1, 0, 2).astype(dt))


def pretile_col(v):
    """[E] -> [128, E//128]: out[p, t] = v[t*128+p]"""
    return np.ascontiguousarray(v.reshape(-1, 128).T.astype(np.float32))


_CACHE = {}


def kernel(**inputs):
    global LAST_RESULT
    inp = {k: np.ascontiguousarray(np.asarray(v, dtype=np.float32))
           for k, v in inputs.items()}
    x = inp['encoder_input']
    s_total = x.shape[0]
    n_cores = 8
    sl = s_total // n_cores

    key = (s_total, n_cores)
    if key not in _CACHE:
        _CACHE[key] = build_nc(s_total=s_total, n_cores=n_cores)
    nc = _CACHE[key]

    xT = np.ascontiguousarray(x.T)
    xTb = xT.astype(ml_dtypes.bfloat16)
    common = {"bv": inp["bv"]}
    common.update({bn: pretile_col(inp[n])
                   for n, bn in (("bq", "bq"), ("bk", "bk"), ("bo", "bo"),
                                 ("b_ff1", "b_ff1"), ("b_ff2", "b_ff2"))})
    common.update({n: pretile_col(inp[n]) for n in ("g1", "b1", "g2", "b2")})
    for n in ("Wq", "Wk", "Wo", "W_ff1"):
        common[n] = pretile_lhsT(inp[n], ml_dtypes.bfloat16)
    common["Wv"] = pretile_rhs(inp["Wv"], 256, ml_dtypes.bfloat16)
    common["W_ff2"] = pretile_wf2(inp["W_ff2"], ml_dtypes.bfloat16)
    in_maps = [{"xT": pretile_x(xT[:, r * sl:(r + 1) * sl], np.float32),
                "xTb": pretile_x(xTb[:, r * sl:(r + 1) * sl],
                                 ml_dtypes.bfloat16),
                **common}
               for r in range(n_cores)]

    res = run_bass_kernel_spmd(nc, in_maps, list(range(n_cores)),
                               trace=TRACE, **TRACE_KWARGS)
    LAST_RESULT = res
    out = np.concatenate([res.results[r]["outT"] for r in range(n_cores)],
                         axis=1).T
    return np.ascontiguousarray(out)
